# revision 2
# baseline (speedup 1.0000x reference)
"""Multi-head attention forward on 8 Trainium2 NeuronCores.

Strategy: pure data-parallel over batch (B=8 -> 1 batch element per core,
no collectives). Per core, one fused kernel computes
    y = softmax((x Wq + bq)(x Wk + bk)^T / sqrt(hd)) (x Wv + bv) @ Wp + bp
for x [1024, 768], H=12 heads of 64 dims.

Layout choices (all matmuls contract over the SBUF partition dim):
  - x^T [768, 1024] built once via DMA-transpose (bf16).
  - Q^T/K^T computed in "dout-major" layout [1536, 1024] (12 tiles of 128
    partitions = 2 heads each).
  - V computed in s-major layout [1024, 12*65] with a constant-1 column per
    head, so each AV matmul also produces the softmax denominator row.
  - scores^T [k, q] per head; exp on ScalarE with the 1/8 scale folded in;
    no max subtraction (scores are O(1) for this distribution).
  - AV: out_h^T [65, q] = V_ext^T @ exp^T accumulated over k tiles; row 64
    holds the softmax sums.
  - normalize: sums broadcast across partitions with a K=1 matmul, then one
    fused DVE divide while draining PSUM -> attn-out^T (bf16).
  - proj: y [s, 768] = attn-out^T^T @ Wp with K=64 chunks + K=1 bias matmul.
Compute dtype bf16 (fp32 PSUM accumulation).
"""

import sys
import types

for _p in ("/opt/trn_rl_repo", "/root/.axon_site/_ro/trn_rl_repo"):
    if _p not in sys.path:
        sys.path.append(_p)

import numpy as np

import concourse.bacc as bacc
import concourse.mybir as mybir
import concourse.tile as tile
from concourse.bass_utils import run_bass_kernel_spmd

N_CORES = 8
P = 128
S = 1024
D = 768
H = 12
HD = 64
ND = D // P            # 6 d_model chunks
NS = S // P            # 8 seq tiles
NM = (2 * D) // P      # 12 M-tiles over Q,K douts
SCALE = 1.0 / (HD ** 0.5)
BF = mybir.dt.bfloat16
F32 = mybir.dt.float32
AF = mybir.ActivationFunctionType
ALU = mybir.AluOpType

_cached = None


def _build():
    nc = bacc.Bacc("TRN2", target_bir_lowering=False, debug=False,
                   enable_asserts=True, num_devices=N_CORES)

    x_ext = nc.dram_tensor("x", [S, D], F32, kind="ExternalInput").ap()
    wq_ext = nc.dram_tensor("W_qkv", [D, 3 * D], F32, kind="ExternalInput").ap()
    bq_ext = nc.dram_tensor("b_qkv", [1, 3 * D], F32, kind="ExternalInput").ap()
    wp_ext = nc.dram_tensor("W_proj", [D, D], F32, kind="ExternalInput").ap()
    bp_ext = nc.dram_tensor("b_proj", [1, D], F32, kind="ExternalInput").ap()
    out_ext = nc.dram_tensor("out", [S, D], F32, kind="ExternalOutput").ap()

    with tile.TileContext(nc) as tc:
        _body(nc, tc, x_ext, wq_ext, bq_ext, wp_ext, bp_ext, out_ext)

    nc.compile()
    return nc


def _body(nc, tc, x_ext, wq_ext, bq_ext, wp_ext, bp_ext, out_ext):
    from contextlib import ExitStack
    with ExitStack() as ctx:
        persist = ctx.enter_context(tc.tile_pool(name="persist", bufs=1))
        xin = ctx.enter_context(tc.tile_pool(name="xin", bufs=1))
        expp = ctx.enter_context(tc.tile_pool(name="expp", bufs=2))
        sums_p = ctx.enter_context(tc.tile_pool(name="sums", bufs=2))
        yout = ctx.enter_context(tc.tile_pool(name="yout", bufs=2))
        ps_mm = ctx.enter_context(tc.tile_pool(name="ps_mm", bufs=2, space="PSUM"))
        ps_sc = ctx.enter_context(tc.tile_pool(name="ps_sc", bufs=2, space="PSUM"))
        ps_av = ctx.enter_context(tc.tile_pool(name="ps_av", bufs=2, space="PSUM"))
        ps_bc = ctx.enter_context(tc.tile_pool(name="ps_bc", bufs=2, space="PSUM"))

        # ---- loads (gpsimd DMA casts f32 -> bf16 in flight) ----
        w_bf = persist.tile([P, ND, 3 * D], BF)
        for kc in range(ND):
            nc.gpsimd.dma_start(w_bf[:, kc, :], wq_ext[kc * P:(kc + 1) * P, :])
        wp_bf = persist.tile([HD, H, D], BF)
        for h in range(H):
            nc.gpsimd.dma_start(wp_bf[:, h, :], wp_ext[h * HD:(h + 1) * HD, :])
        x_bf = xin.tile([P, NS, D], BF)
        for sb in range(NS):
            nc.gpsimd.dma_start(x_bf[:, sb, :], x_ext[sb * P:(sb + 1) * P, :])

        bqkT = persist.tile([P, NM], F32)   # col m = b_qkv[m*128:(m+1)*128]
        for m in range(NM):
            nc.sync.dma_start(bqkT[:, m:m + 1], bq_ext[0:1, m * P:(m + 1) * P])
        bv_bf = persist.tile([1, D], BF)
        nc.gpsimd.dma_start(bv_bf, bq_ext[0:1, 2 * D:3 * D])
        bp_bf = persist.tile([1, D], BF)
        nc.gpsimd.dma_start(bp_bf, bp_ext[0:1, :])
        ones1 = persist.tile([1, P], BF)
        nc.vector.memset(ones1, 1.0)
        # ones row living at partition 64 (for the sums-broadcast matmul whose
        # rhs is PSUM row 64 copied to SBUF partition 64)
        ones64 = persist.tile([65, HD], BF)
        nc.vector.memset(ones64[64:65, :], 1.0)

        # ---- x^T via DMA transpose (SBUF -> SBUF, bf16) ----
        xT = persist.tile([P, ND, S], BF)
        for kc in range(ND):
            for sb in range(NS):
                nc.sync.dma_start(xT[:, kc, sb * P:(sb + 1) * P],
                                  x_bf[:, sb, kc * P:(kc + 1) * P],
                                  transpose=True)

        # ---- Q^T / K^T : qkT[:, m, :] = (x @ Wqkv[:, m*128:+128])^T + b ----
        qkT = persist.tile([P, NM, S], BF)
        for m in range(NM):
            for nh in range(2):
                ps = ps_mm.tile([P, 512], F32)
                for kc in range(ND):
                    nc.tensor.matmul(ps,
                                     w_bf[:, kc, m * P:(m + 1) * P],
                                     xT[:, kc, nh * 512:(nh + 1) * 512],
                                     start=(kc == 0), stop=(kc == ND - 1))
                nc.vector.tensor_scalar(
                    out=qkT[:, m, nh * 512:(nh + 1) * 512], in0=ps,
                    scalar1=bqkT[:, m:m + 1], scalar2=None, op0=ALU.add)

        # ---- V in s-major with ones column per head: [s, 12, 65] ----
        vext = persist.tile([P, NS, H * 65], BF)
        nc.vector.memset(vext, 1.0)
        for sb in range(NS):
            for c0, cn in ((0, 512), (512, 256)):
                ps = ps_mm.tile([P, 512], F32)
                for kc in range(ND):
                    nc.tensor.matmul(ps[:, :cn],
                                     xT[:, kc, sb * P:(sb + 1) * P],
                                     w_bf[:, kc, 2 * D + c0:2 * D + c0 + cn],
                                     start=(kc == 0), stop=False)
                nc.tensor.matmul(ps[:, :cn], ones1, bv_bf[:, c0:c0 + cn],
                                 start=False, stop=True)
                h0 = c0 // HD
                nh_h = cn // HD
                src = ps[:, :cn].rearrange("p (h c) -> p h c", c=HD)
                dst = vext[:, sb, :].rearrange("p (h c) -> p h c", c=65)
                nc.vector.tensor_copy(dst[:, h0:h0 + nh_h, 0:HD], src)

        # ---- attention per head ----
        aoT = persist.tile([HD, H, S], BF)   # attn output, transposed, per head
        for h in range(H):
            g, half = divmod(h, 2)
            qp = slice(half * HD, (half + 1) * HD)
            expT = expp.tile([P, NS, S], BF, tag="expT")
            for kb in range(NS):
                for qh in range(2):
                    ps = ps_sc.tile([P, 512], F32)
                    nc.tensor.matmul(ps,
                                     qkT[qp, ND + g, kb * P:(kb + 1) * P],
                                     qkT[qp, g, qh * 512:(qh + 1) * 512],
                                     start=True, stop=True)
                    nc.scalar.activation(expT[:, kb, qh * 512:(qh + 1) * 512],
                                         ps, AF.Exp, scale=SCALE)
            lnz = sums_p.tile([65, S], F32, tag="lnz")
            recbf = sums_p.tile([65, S], BF, tag="recbf")
            for qh in range(2):
                po = ps_av.tile([65, 512], F32, tag="ps_av")
                for kb in range(NS):
                    nc.tensor.matmul(po,
                                     vext[:, kb, h * 65:(h + 1) * 65],
                                     expT[:, kb, qh * 512:(qh + 1) * 512],
                                     start=(kb == 0), stop=(kb == NS - 1))
                # softmax sums in row 64 -> 1/Z = exp(-ln Z) on ScalarE
                qs = slice(qh * 512, (qh + 1) * 512)
                nc.scalar.activation(lnz[64:65, qs], po[64:65, :], AF.Ln)
                nc.scalar.activation(recbf[64:65, qs], lnz[64:65, qs],
                                     AF.Exp, scale=-1.0)
                # broadcast 1/Z across 64 partitions via K=1 matmul
                pb = ps_bc.tile([HD, 512], F32, tag="ps_bc")
                nc.tensor.matmul(pb, ones64[64:65, :], recbf[64:65, qs],
                                 start=True, stop=True)
                nc.vector.tensor_copy(aoT[:, h, qs], po[0:64, :])
                nc.vector.tensor_mul(aoT[:, h, qs], aoT[:, h, qs], pb)

        # ---- output projection ----
        for sb in range(NS):
            y_sb = yout.tile([P, D], F32, tag="y")
            for c0, cn in ((0, 512), (512, 256)):
                ps = ps_mm.tile([P, 512], F32)
                for h in range(H):
                    nc.tensor.matmul(ps[:, :cn],
                                     aoT[:, h, sb * P:(sb + 1) * P],
                                     wp_bf[:, h, c0:c0 + cn],
                                     start=(h == 0), stop=False)
                nc.tensor.matmul(ps[:, :cn], ones1, bp_bf[:, c0:c0 + cn],
                                 start=False, stop=True)
                nc.vector.tensor_copy(y_sb[:, c0:c0 + cn], ps[:, :cn])
            nc.sync.dma_start(out_ext[sb * P:(sb + 1) * P, :], y_sb)


def kernel(**inputs):
    global _cached
    x = np.ascontiguousarray(np.asarray(inputs["x"], dtype=np.float32))
    w_qkv = np.ascontiguousarray(np.asarray(inputs["W_qkv"], dtype=np.float32))
    b_qkv = np.ascontiguousarray(np.asarray(inputs["b_qkv"], dtype=np.float32)).reshape(1, -1)
    w_proj = np.ascontiguousarray(np.asarray(inputs["W_proj"], dtype=np.float32))
    b_proj = np.ascontiguousarray(np.asarray(inputs["b_proj"], dtype=np.float32)).reshape(1, -1)

    if _cached is None:
        _cached = _build()
    nc = _cached

    in_maps = [{"x": x[b], "W_qkv": w_qkv, "b_qkv": b_qkv,
                "W_proj": w_proj, "b_proj": b_proj} for b in range(N_CORES)]
    res = run_bass_kernel_spmd(nc, in_maps, core_ids=list(range(N_CORES)))
    return np.stack([res.results[i]["out"] for i in range(N_CORES)], axis=0)


# revision 6
# speedup vs baseline: 1.3240x; 1.3240x over previous
"""Multi-head attention forward on 8 Trainium2 NeuronCores.

Strategy: pure data-parallel over batch (B=8 -> 1 batch element per core,
no collectives). Per core, one fused kernel computes
    y = softmax((x Wq + bq)(x Wk + bk)^T / sqrt(hd)) (x Wv + bv) @ Wp + bp
for x [1024, 768], H=12 heads of 64 dims.

Layout choices (all matmuls contract over the SBUF partition dim):
  - x^T [768, 1024] built from f32 x via PE transposes, cast to bf16 in the
    PSUM drain.
  - Q^T/K^T computed in "dout-major" layout [1536, 1024] (12 tiles of 128
    partitions = 2 heads each), interleaved m-order so head 0 unblocks early.
  - V computed in s-major layout [1024, 12*65] with a constant-1 column per
    head, so each AV matmul also produces the softmax denominator row.
  - scores^T [k, q] per head; exp on ScalarE with the 1/8 scale folded in;
    no max subtraction (scores are O(1) for this distribution).
  - AV: out_h^T [65, q] = V_ext^T @ exp^T accumulated over k tiles; row 64
    holds the softmax sums Z; 1/Z = exp(-ln Z) on ScalarE, broadcast across
    64 partitions with a K=1 matmul, applied with one DVE multiply.
  - proj: y [s, 768] = attn-out^T^T @ Wp with K=64 chunks + K=1 bias matmul.
Compute dtype bf16 (fp32 PSUM accumulation).
"""

import sys

for _p in ("/opt/trn_rl_repo", "/root/.axon_site/_ro/trn_rl_repo"):
    if _p not in sys.path:
        sys.path.append(_p)

import numpy as np

import concourse.bacc as bacc
import concourse.mybir as mybir
import concourse.tile as tile
from concourse.bass_utils import run_bass_kernel_spmd
from concourse.masks import make_identity

N_CORES = 8
P = 128
S = 1024
D = 768
H = 12
HD = 64
ND = D // P            # 6 d_model chunks
NS = S // P            # 8 seq tiles
NM = (2 * D) // P      # 12 M-tiles over Q,K douts
SCALE = 1.0 / (HD ** 0.5)
BF = mybir.dt.bfloat16
F32 = mybir.dt.float32
AF = mybir.ActivationFunctionType
ALU = mybir.AluOpType

_cached = None


def _build():
    nc = bacc.Bacc("TRN2", target_bir_lowering=False, debug=False,
                   enable_asserts=True, num_devices=N_CORES)

    x_ext = nc.dram_tensor("x", [S, D], F32, kind="ExternalInput").ap()
    wq_ext = nc.dram_tensor("W_qkv", [D, 3 * D], F32, kind="ExternalInput").ap()
    bq_ext = nc.dram_tensor("b_qkv", [1, 3 * D], F32, kind="ExternalInput").ap()
    wp_ext = nc.dram_tensor("W_proj", [D, D], F32, kind="ExternalInput").ap()
    bp_ext = nc.dram_tensor("b_proj", [1, D], F32, kind="ExternalInput").ap()
    out_ext = nc.dram_tensor("out", [S, D], F32, kind="ExternalOutput").ap()

    with tile.TileContext(nc) as tc:
        _body(nc, tc, x_ext, wq_ext, bq_ext, wp_ext, bp_ext, out_ext)

    nc.compile()
    return nc


def _body(nc, tc, x_ext, wq_ext, bq_ext, wp_ext, bp_ext, out_ext):
    from contextlib import ExitStack
    with ExitStack() as ctx:
        persist = ctx.enter_context(tc.tile_pool(name="persist", bufs=1))
        xin = ctx.enter_context(tc.tile_pool(name="xin", bufs=4))
        expp = ctx.enter_context(tc.tile_pool(name="expp", bufs=2))
        sums_p = ctx.enter_context(tc.tile_pool(name="sums", bufs=2))
        yout = ctx.enter_context(tc.tile_pool(name="yout", bufs=2))
        ps_mm = ctx.enter_context(tc.tile_pool(name="ps_mm", bufs=2, space="PSUM"))
        ps_sc = ctx.enter_context(tc.tile_pool(name="ps_sc", bufs=2, space="PSUM"))
        ps_av = ctx.enter_context(tc.tile_pool(name="ps_av", bufs=1, space="PSUM"))

        # ---- loads ----
        # x: f32 via HWDGE (fast ring), transposed on the PE, cast in drain
        ident = persist.tile([P, P], F32)
        make_identity(nc, ident)
        xT = persist.tile([P, ND, S], BF)
        for sb in range(NS):
            x_f = xin.tile([P, D], F32, tag="x_f")
            nc.sync.dma_start(x_f, x_ext[sb * P:(sb + 1) * P, :])
            for kc in range(ND):
                pt_full = ps_mm.tile([P, 512], F32, tag="ps_mm")
                pt = pt_full[:, 0:P]
                nc.tensor.transpose(pt, x_f[:, kc * P:(kc + 1) * P], ident)
                nc.vector.tensor_copy(xT[:, kc, sb * P:(sb + 1) * P], pt)

        # weights: gpsimd cast-DMAs f32 -> bf16
        w_bf = persist.tile([P, ND, 3 * D], BF)
        for kc in range(ND):
            nc.gpsimd.dma_start(w_bf[:, kc, :], wq_ext[kc * P:(kc + 1) * P, :])
        wp_bf = persist.tile([HD, H, D], BF)
        for h in range(H):
            nc.gpsimd.dma_start(wp_bf[:, h, :], wp_ext[h * HD:(h + 1) * HD, :])

        bqkT = persist.tile([P, NM], F32)   # col m = b_qkv[m*128:(m+1)*128]
        for m in range(NM):
            nc.sync.dma_start(bqkT[:, m:m + 1], bq_ext[0:1, m * P:(m + 1) * P])
        bv_bf = persist.tile([1, D], BF)
        nc.gpsimd.dma_start(bv_bf, bq_ext[0:1, 2 * D:3 * D])
        bp_bf = persist.tile([1, D], BF)
        nc.gpsimd.dma_start(bp_bf, bp_ext[0:1, :])
        ones1 = persist.tile([1, P], BF)
        nc.vector.memset(ones1, 1.0)
        ones64 = persist.tile([65, HD], BF)
        nc.vector.memset(ones64[64:65, :], 1.0)
        vext = persist.tile([P, NS, H * 65], BF)
        nc.vector.memset(vext, 1.0)

        # ---- Q^T / K^T, interleaved so head-pair g is ready after 2 tiles ----
        qkT = persist.tile([P, NM, S], BF)
        m_order = [m for g in range(ND) for m in (g, ND + g)]
        for m in m_order:
            for nh in range(2):
                ps = ps_mm.tile([P, 512], F32, tag="ps_mm")
                for kc in range(ND):
                    nc.tensor.matmul(ps,
                                     w_bf[:, kc, m * P:(m + 1) * P],
                                     xT[:, kc, nh * 512:(nh + 1) * 512],
                                     start=(kc == 0), stop=(kc == ND - 1))
                nc.vector.tensor_scalar(
                    out=qkT[:, m, nh * 512:(nh + 1) * 512], in0=ps,
                    scalar1=bqkT[:, m:m + 1], scalar2=None, op0=ALU.add)

        # ---- V in s-major with ones column per head: [s, 12, 65] ----
        for sb in range(NS):
            for c0, cn in ((0, 512), (512, 256)):
                ps = ps_mm.tile([P, 512], F32, tag="ps_mm")
                for kc in range(ND):
                    nc.tensor.matmul(ps[:, :cn],
                                     xT[:, kc, sb * P:(sb + 1) * P],
                                     w_bf[:, kc, 2 * D + c0:2 * D + c0 + cn],
                                     start=(kc == 0), stop=False)
                nc.tensor.matmul(ps[:, :cn], ones1, bv_bf[:, c0:c0 + cn],
                                 start=False, stop=True)
                h0 = c0 // HD
                nh_h = cn // HD
                src = ps[:, :cn].rearrange("p (h c) -> p h c", c=HD)
                dst = vext[:, sb, :].rearrange("p (h c) -> p h c", c=65)
                nc.vector.tensor_copy(dst[:, h0:h0 + nh_h, 0:HD], src)

        # ---- attention per head ----
        aoT = persist.tile([HD, H, S], BF)   # attn output, transposed, per head
        for h in range(H):
            g, half = divmod(h, 2)
            qp = slice(half * HD, (half + 1) * HD)
            expT = expp.tile([P, NS, S], BF, tag="expT")
            for kb in range(NS):
                ps = ps_sc.tile([P, S], F32, tag="ps_sc")
                for qh in range(2):
                    nc.tensor.matmul(ps[:, qh * 512:(qh + 1) * 512],
                                     qkT[qp, ND + g, kb * P:(kb + 1) * P],
                                     qkT[qp, g, qh * 512:(qh + 1) * 512],
                                     start=True, stop=True)
                nc.scalar.activation(expT[:, kb, :], ps, AF.Exp, scale=SCALE)
            lnz = sums_p.tile([65, S], F32, tag="lnz")
            recbf = sums_p.tile([65, S], BF, tag="recbf")
            po = ps_av.tile([65, S], F32, tag="ps_av")
            for qh in range(2):
                for kb in range(NS):
                    nc.tensor.matmul(po[:, qh * 512:(qh + 1) * 512],
                                     vext[:, kb, h * 65:(h + 1) * 65],
                                     expT[:, kb, qh * 512:(qh + 1) * 512],
                                     start=(kb == 0), stop=(kb == NS - 1))
            # softmax sums in row 64 -> 1/Z = exp(-ln Z) on ScalarE
            nc.scalar.activation(lnz[64:65, :], po[64:65, :], AF.Ln)
            nc.scalar.activation(recbf[64:65, :], lnz[64:65, :],
                                 AF.Exp, scale=-1.0)
            for qh in range(2):
                qs = slice(qh * 512, (qh + 1) * 512)
                pb_full = ps_mm.tile([P, 512], F32, tag="ps_mm")
                pb = pb_full[0:HD, :]
                nc.tensor.matmul(pb, ones64[64:65, :], recbf[64:65, qs],
                                 start=True, stop=True)
                nc.vector.tensor_copy(aoT[:, h, qs], po[0:64, qs])
                nc.vector.tensor_mul(aoT[:, h, qs], aoT[:, h, qs], pb)

        # ---- output projection ----
        for sb in range(NS):
            y_sb = yout.tile([P, D], F32, tag="y")
            for c0, cn in ((0, 512), (512, 256)):
                ps = ps_mm.tile([P, 512], F32, tag="ps_mm")
                for h in range(H):
                    nc.tensor.matmul(ps[:, :cn],
                                     aoT[:, h, sb * P:(sb + 1) * P],
                                     wp_bf[:, h, c0:c0 + cn],
                                     start=(h == 0), stop=False)
                nc.tensor.matmul(ps[:, :cn], ones1, bp_bf[:, c0:c0 + cn],
                                 start=False, stop=True)
                nc.vector.tensor_copy(y_sb[:, c0:c0 + cn], ps[:, :cn])
            nc.sync.dma_start(out_ext[sb * P:(sb + 1) * P, :], y_sb)


def kernel(**inputs):
    global _cached
    x = np.ascontiguousarray(np.asarray(inputs["x"], dtype=np.float32))
    w_qkv = np.ascontiguousarray(np.asarray(inputs["W_qkv"], dtype=np.float32))
    b_qkv = np.ascontiguousarray(np.asarray(inputs["b_qkv"], dtype=np.float32)).reshape(1, -1)
    w_proj = np.ascontiguousarray(np.asarray(inputs["W_proj"], dtype=np.float32))
    b_proj = np.ascontiguousarray(np.asarray(inputs["b_proj"], dtype=np.float32)).reshape(1, -1)

    if _cached is None:
        _cached = _build()
    nc = _cached

    in_maps = [{"x": x[b], "W_qkv": w_qkv, "b_qkv": b_qkv,
                "W_proj": w_proj, "b_proj": b_proj} for b in range(N_CORES)]
    res = run_bass_kernel_spmd(nc, in_maps, core_ids=list(range(N_CORES)))
    return np.stack([res.results[i]["out"] for i in range(N_CORES)], axis=0)


# revision 11
# speedup vs baseline: 1.4453x; 1.0916x over previous
"""Multi-head attention forward on 8 Trainium2 NeuronCores.

Strategy: pure data-parallel over batch (B=8 -> 1 batch element per core,
no collectives). Per core, one fused kernel computes
    y = softmax((x Wq + bq)(x Wk + bk)^T / sqrt(hd)) (x Wv + bv) @ Wp + bp
for x [1024, 768], H=12 heads of 64 dims.

Layout choices (all matmuls contract over the SBUF partition dim):
  - x^T [768, 1024] built from f32 x via PE transposes, cast to bf16 in the
    PSUM drain.
  - Q^T/K^T computed in "dout-major" layout [1536, 1024] (12 tiles of 128
    partitions = 2 heads each), interleaved m-order so head 0 unblocks early.
  - V computed in s-major layout [1024, 12*65] with a constant-1 column per
    head, so each AV matmul also produces the softmax denominator row.
  - scores^T [k, q] per head; exp on ScalarE with the 1/8 scale folded in;
    no max subtraction (scores are O(1) for this distribution).
  - AV: out_h^T [65, q] = V_ext^T @ exp^T accumulated over k tiles; row 64
    holds the softmax sums Z; 1/Z = exp(-ln Z) on ScalarE, broadcast across
    64 partitions with a K=1 matmul, applied with one DVE multiply.
  - proj: y [s, 768] = attn-out^T^T @ Wp with K=64 chunks + K=1 bias matmul.
Compute dtype bf16 (fp32 PSUM accumulation).
"""

import sys

for _p in ("/opt/trn_rl_repo", "/root/.axon_site/_ro/trn_rl_repo"):
    if _p not in sys.path:
        sys.path.append(_p)

import numpy as np

import concourse.bacc as bacc
import concourse.mybir as mybir
import concourse.tile as tile
from concourse.bass_utils import run_bass_kernel_spmd
from concourse.masks import make_identity

N_CORES = 8
P = 128
S = 1024
D = 768
H = 12
HD = 64
ND = D // P            # 6 d_model chunks
NS = S // P            # 8 seq tiles
NM = (2 * D) // P      # 12 M-tiles over Q,K douts
SCALE = 1.0 / (HD ** 0.5)
BF = mybir.dt.bfloat16
F32 = mybir.dt.float32
AF = mybir.ActivationFunctionType
ALU = mybir.AluOpType

_cached = None


def _patch_act_tables():
    """Force every Exp/Ln activation onto the one table set that holds both
    (`natural_log_exp_and_others`), so the table is loaded once instead of
    thrashing between `exp_and_others` and the ln set on every head."""
    import concourse.bacc as _bacc
    if getattr(_bacc, "_act_tables_patched", False):
        return
    orig = _bacc.get_activation_tables

    def patched(arch):
        tables = dict(orig(arch))
        for name, fns in tables.items():
            if name != "natural_log_exp_and_others":
                tables[name] = fns - {AF.Exp, AF.Ln}
        return tables

    _bacc.get_activation_tables = patched
    _bacc._act_tables_patched = True


def _build():
    _patch_act_tables()
    nc = bacc.Bacc("TRN2", target_bir_lowering=False, debug=False,
                   enable_asserts=True, num_devices=N_CORES)

    x_ext = nc.dram_tensor("x", [S, D], F32, kind="ExternalInput").ap()
    wq_ext = nc.dram_tensor("W_qkv", [D, 3 * D], F32, kind="ExternalInput").ap()
    bq_ext = nc.dram_tensor("b_qkv", [1, 3 * D], F32, kind="ExternalInput").ap()
    wp_ext = nc.dram_tensor("W_proj", [D, D], F32, kind="ExternalInput").ap()
    bp_ext = nc.dram_tensor("b_proj", [1, D], F32, kind="ExternalInput").ap()
    out_ext = nc.dram_tensor("out", [S, D], F32, kind="ExternalOutput").ap()

    with tile.TileContext(nc) as tc:
        _body(nc, tc, x_ext, wq_ext, bq_ext, wp_ext, bp_ext, out_ext)

    nc.compile()
    return nc


def _body(nc, tc, x_ext, wq_ext, bq_ext, wp_ext, bp_ext, out_ext):
    from contextlib import ExitStack
    with ExitStack() as ctx:
        persist = ctx.enter_context(tc.tile_pool(name="persist", bufs=1))
        xin = ctx.enter_context(tc.tile_pool(name="xin", bufs=4))
        expp = ctx.enter_context(tc.tile_pool(name="expp", bufs=2))
        sums_p = ctx.enter_context(tc.tile_pool(name="sums", bufs=2))
        yout = ctx.enter_context(tc.tile_pool(name="yout", bufs=2))
        ps_mm = ctx.enter_context(tc.tile_pool(name="ps_mm", bufs=2, space="PSUM"))

        # ---- loads ----
        # x: f32 via HWDGE (fast ring), transposed on the PE, cast in drain
        ident = persist.tile([P, P], F32)
        make_identity(nc, ident)
        xT = persist.tile([P, ND, S], BF)
        with tc.tile_pool(name="ps_tr", bufs=4, space="PSUM") as ps_tr:
            for sb in range(NS):
                x_f = xin.tile([P, D], F32, tag="x_f")
                nc.sync.dma_start(x_f, x_ext[sb * P:(sb + 1) * P, :])
                for kc in range(ND):
                    pt = ps_tr.tile([P, P], F32, tag="ps_tr")
                    nc.tensor.transpose(pt, x_f[:, kc * P:(kc + 1) * P], ident)
                    nc.vector.tensor_copy(xT[:, kc, sb * P:(sb + 1) * P], pt)
        ps_sc = ctx.enter_context(tc.tile_pool(name="ps_sc", bufs=2, space="PSUM"))
        ps_av = ctx.enter_context(tc.tile_pool(name="ps_av", bufs=1, space="PSUM"))

        # weights: gpsimd cast-DMAs f32 -> bf16
        w_bf = persist.tile([P, ND, 3 * D], BF)
        for kc in range(ND):
            nc.gpsimd.dma_start(w_bf[:, kc, :], wq_ext[kc * P:(kc + 1) * P, :])
        wp_bf = persist.tile([HD, H, D], BF)
        for h in range(H):
            nc.gpsimd.dma_start(wp_bf[:, h, :], wp_ext[h * HD:(h + 1) * HD, :])

        bqkT = persist.tile([P, NM], F32)   # col m = b_qkv[m*128:(m+1)*128]
        for m in range(NM):
            nc.sync.dma_start(bqkT[:, m:m + 1], bq_ext[0:1, m * P:(m + 1) * P])
        bv_bf = persist.tile([1, D], BF)
        nc.gpsimd.dma_start(bv_bf, bq_ext[0:1, 2 * D:3 * D])
        bp_bf = persist.tile([1, D], BF)
        nc.gpsimd.dma_start(bp_bf, bp_ext[0:1, :])
        ones1 = persist.tile([1, P], BF)
        nc.vector.memset(ones1, 1.0)
        ones64 = persist.tile([65, HD], BF)
        nc.vector.memset(ones64[64:65, :], 1.0)
        vext = persist.tile([P, NS, H * 65], BF)
        for sb in range(NS):
            dst = vext[:, sb, :].rearrange("p (h c) -> p h c", c=65)
            nc.vector.memset(dst[:, :, 64:65], 1.0)

        # ---- Q^T / K^T, interleaved so head-pair g is ready after 2 tiles ----
        qkT = persist.tile([P, NM, S], BF)
        m_order = [m for g in range(ND) for m in (g, ND + g)]
        for m in m_order:
            for nh in range(2):
                ps = ps_mm.tile([P, 512], F32, tag="ps_mm")
                for kc in range(ND):
                    nc.tensor.matmul(ps,
                                     w_bf[:, kc, m * P:(m + 1) * P],
                                     xT[:, kc, nh * 512:(nh + 1) * 512],
                                     start=(kc == 0), stop=(kc == ND - 1))
                nc.vector.tensor_scalar(
                    out=qkT[:, m, nh * 512:(nh + 1) * 512], in0=ps,
                    scalar1=bqkT[:, m:m + 1], scalar2=None, op0=ALU.add)

        # ---- V in s-major with ones column per head: [s, 12, 65] ----
        for sb in range(NS):
            for c0, cn in ((0, 512), (512, 256)):
                ps = ps_mm.tile([P, 512], F32, tag="ps_mm")
                for kc in range(ND):
                    nc.tensor.matmul(ps[:, :cn],
                                     xT[:, kc, sb * P:(sb + 1) * P],
                                     w_bf[:, kc, 2 * D + c0:2 * D + c0 + cn],
                                     start=(kc == 0), stop=False)
                nc.tensor.matmul(ps[:, :cn], ones1, bv_bf[:, c0:c0 + cn],
                                 start=False, stop=True)
                h0 = c0 // HD
                nh_h = cn // HD
                src = ps[:, :cn].rearrange("p (h c) -> p h c", c=HD)
                dst = vext[:, sb, :].rearrange("p (h c) -> p h c", c=65)
                nc.vector.tensor_copy(dst[:, h0:h0 + nh_h, 0:HD], src)

        # ---- attention per head ----
        aoT = persist.tile([HD, H, S], BF)   # attn output, transposed, per head
        for h in range(H):
            g, half = divmod(h, 2)
            qp = slice(half * HD, (half + 1) * HD)
            expT = expp.tile([P, NS, S], BF, tag="expT")
            for kb in range(NS):
                ps = ps_sc.tile([P, S], F32, tag="ps_sc")
                for qh in range(2):
                    nc.tensor.matmul(ps[:, qh * 512:(qh + 1) * 512],
                                     qkT[qp, ND + g, kb * P:(kb + 1) * P],
                                     qkT[qp, g, qh * 512:(qh + 1) * 512],
                                     start=True, stop=True)
                nc.scalar.activation(expT[:, kb, :], ps, AF.Exp, scale=SCALE)
            lnz = sums_p.tile([65, S], F32, tag="lnz")
            recbf = sums_p.tile([65, S], BF, tag="recbf")
            po = ps_av.tile([65, S], F32, tag="ps_av")
            for qh in range(2):
                for kb in range(NS):
                    nc.tensor.matmul(po[:, qh * 512:(qh + 1) * 512],
                                     vext[:, kb, h * 65:(h + 1) * 65],
                                     expT[:, kb, qh * 512:(qh + 1) * 512],
                                     start=(kb == 0), stop=(kb == NS - 1))
            # softmax sums in row 64 -> 1/Z = exp(-ln Z) on ScalarE (same
            # ACT table set as the scores exp, see _patch_act_tables)
            nc.scalar.activation(lnz[64:65, :], po[64:65, :], AF.Ln)
            nc.scalar.activation(recbf[64:65, :], lnz[64:65, :],
                                 AF.Exp, scale=-1.0)
            for qh in range(2):
                qs = slice(qh * 512, (qh + 1) * 512)
                pb_full = ps_mm.tile([P, 512], F32, tag="ps_mm")
                pb = pb_full[0:HD, :]
                nc.tensor.matmul(pb, ones64[64:65, :], recbf[64:65, qs],
                                 start=True, stop=True)
                nc.vector.tensor_copy(aoT[:, h, qs], po[0:64, qs])
                nc.vector.tensor_mul(aoT[:, h, qs], aoT[:, h, qs], pb)

        # ---- output projection ----
        for sb in range(NS):
            y_sb = yout.tile([P, D], F32, tag="y")
            for c0, cn in ((0, 512), (512, 256)):
                ps = ps_mm.tile([P, 512], F32, tag="ps_mm")
                for h in range(H):
                    nc.tensor.matmul(ps[:, :cn],
                                     aoT[:, h, sb * P:(sb + 1) * P],
                                     wp_bf[:, h, c0:c0 + cn],
                                     start=(h == 0), stop=False)
                nc.tensor.matmul(ps[:, :cn], ones1, bp_bf[:, c0:c0 + cn],
                                 start=False, stop=True)
                nc.vector.tensor_copy(y_sb[:, c0:c0 + cn], ps[:, :cn])
            nc.sync.dma_start(out_ext[sb * P:(sb + 1) * P, :], y_sb)


def kernel(**inputs):
    global _cached
    x = np.ascontiguousarray(np.asarray(inputs["x"], dtype=np.float32))
    w_qkv = np.ascontiguousarray(np.asarray(inputs["W_qkv"], dtype=np.float32))
    b_qkv = np.ascontiguousarray(np.asarray(inputs["b_qkv"], dtype=np.float32)).reshape(1, -1)
    w_proj = np.ascontiguousarray(np.asarray(inputs["W_proj"], dtype=np.float32))
    b_proj = np.ascontiguousarray(np.asarray(inputs["b_proj"], dtype=np.float32)).reshape(1, -1)

    if _cached is None:
        _cached = _build()
    nc = _cached

    in_maps = [{"x": x[b], "W_qkv": w_qkv, "b_qkv": b_qkv,
                "W_proj": w_proj, "b_proj": b_proj} for b in range(N_CORES)]
    res = run_bass_kernel_spmd(nc, in_maps, core_ids=list(range(N_CORES)))
    return np.stack([res.results[i]["out"] for i in range(N_CORES)], axis=0)


# revision 14
# speedup vs baseline: 2.0250x; 1.4011x over previous
"""Multi-head attention forward on 8 Trainium2 NeuronCores.

Strategy: pure data-parallel over batch (B=8 -> 1 batch element per core,
no collectives). Per core, one fused kernel computes
    y = softmax((x Wq + bq)(x Wk + bk)^T / sqrt(hd)) (x Wv + bv) @ Wp + bp
for x [1024, 768], H=12 heads of 64 dims.

Layout choices (all matmuls contract over the SBUF partition dim):
  - x^T [768, 1024] built from f32 x via PE transposes, cast to bf16 in the
    PSUM drain.
  - Q^T/K^T computed in "dout-major" layout [1536, 1024] (12 tiles of 128
    partitions = 2 heads each), interleaved m-order so head 0 unblocks early.
  - V computed in s-major layout [1024, 12*65] with a constant-1 column per
    head, so each AV matmul also produces the softmax denominator row.
  - scores^T [k, q] per head; exp on ScalarE with the 1/8 scale folded in;
    no max subtraction (scores are O(1) for this distribution).
  - AV: out_h^T [65, q] = V_ext^T @ exp^T accumulated over k tiles; row 64
    holds the softmax sums Z; 1/Z = exp(-ln Z) on ScalarE, broadcast across
    64 partitions with a K=1 matmul, applied with one DVE multiply.
  - proj: y [s, 768] = attn-out^T^T @ Wp with K=64 chunks + K=1 bias matmul.
Compute dtype bf16 (fp32 PSUM accumulation).
"""

import sys

for _p in ("/opt/trn_rl_repo", "/root/.axon_site/_ro/trn_rl_repo"):
    if _p not in sys.path:
        sys.path.append(_p)

import numpy as np

import concourse.bacc as bacc
import concourse.mybir as mybir
import concourse.tile as tile
from concourse.bass_utils import run_bass_kernel_spmd
from concourse.masks import make_identity

N_CORES = 8
P = 128
S = 1024
D = 768
H = 12
HD = 64
ND = D // P            # 6 d_model chunks
NS = S // P            # 8 seq tiles
NM = (2 * D) // P      # 12 M-tiles over Q,K douts
SCALE = 1.0 / (HD ** 0.5)
BF = mybir.dt.bfloat16
F32 = mybir.dt.float32
AF = mybir.ActivationFunctionType
ALU = mybir.AluOpType

_cached = None


def _patch_act_tables():
    """Force every Exp/Ln activation onto the one table set that holds both
    (`natural_log_exp_and_others`), so the table is loaded once instead of
    thrashing between `exp_and_others` and the ln set on every head."""
    import concourse.bacc as _bacc
    if getattr(_bacc, "_act_tables_patched", False):
        return
    orig = _bacc.get_activation_tables

    def patched(arch):
        tables = dict(orig(arch))
        for name, fns in tables.items():
            if name != "natural_log_exp_and_others":
                tables[name] = fns - {AF.Exp, AF.Ln}
        return tables

    _bacc.get_activation_tables = patched
    _bacc._act_tables_patched = True


def _build():
    _patch_act_tables()
    nc = bacc.Bacc("TRN2", target_bir_lowering=False, debug=False,
                   enable_asserts=True, num_devices=N_CORES)

    x_ext = nc.dram_tensor("x", [S, D], F32, kind="ExternalInput").ap()
    wq_ext = nc.dram_tensor("W_qkv", [D, 3 * D], F32, kind="ExternalInput").ap()
    bq_ext = nc.dram_tensor("b_qkv", [1, 3 * D], F32, kind="ExternalInput").ap()
    wp_ext = nc.dram_tensor("W_proj", [D, D], F32, kind="ExternalInput").ap()
    bp_ext = nc.dram_tensor("b_proj", [1, D], F32, kind="ExternalInput").ap()
    out_ext = nc.dram_tensor("out", [S, D], F32, kind="ExternalOutput").ap()

    with tile.TileContext(nc) as tc:
        _body(nc, tc, x_ext, wq_ext, bq_ext, wp_ext, bp_ext, out_ext)

    nc.compile()
    return nc


def _body(nc, tc, x_ext, wq_ext, bq_ext, wp_ext, bp_ext, out_ext):
    from contextlib import ExitStack
    with ExitStack() as ctx:
        persist = ctx.enter_context(tc.tile_pool(name="persist", bufs=1))
        yout = ctx.enter_context(tc.tile_pool(name="yout", bufs=2))
        ps_mm = ctx.enter_context(tc.tile_pool(name="ps_mm", bufs=2, space="PSUM"))

        # ---- loads ----
        # x: f32 via HWDGE (fast ring), transposed on the PE, cast in drain
        ident = persist.tile([P, P], F32)
        make_identity(nc, ident)
        xT = persist.tile([P, ND, S], BF)
        with tc.tile_pool(name="xin", bufs=4) as xin, \
             tc.tile_pool(name="ps_tr", bufs=4, space="PSUM") as ps_tr:
            for sb in range(NS):
                x_f = xin.tile([P, D], F32, tag="x_f")
                nc.sync.dma_start(x_f, x_ext[sb * P:(sb + 1) * P, :])
                for kc in range(ND):
                    pt = ps_tr.tile([P, P], F32, tag="ps_tr")
                    nc.tensor.transpose(pt, x_f[:, kc * P:(kc + 1) * P], ident)
                    nc.vector.tensor_copy(xT[:, kc, sb * P:(sb + 1) * P], pt)
        expp = ctx.enter_context(tc.tile_pool(name="expp", bufs=24))
        sums_p = ctx.enter_context(tc.tile_pool(name="sums", bufs=4))
        ps_sc = ctx.enter_context(tc.tile_pool(name="ps_sc", bufs=2, space="PSUM"))
        ps_av = ctx.enter_context(tc.tile_pool(name="ps_av", bufs=2, space="PSUM"))

        # weights: gpsimd cast-DMAs f32 -> bf16
        w_bf = persist.tile([P, ND, 3 * D], BF)
        for kc in range(ND):
            nc.gpsimd.dma_start(w_bf[:, kc, :], wq_ext[kc * P:(kc + 1) * P, :])
        wp_bf = persist.tile([P, ND, D], BF)     # row chunk g = head pair g
        for g in range(ND):
            nc.gpsimd.dma_start(wp_bf[:, g, :], wp_ext[g * P:(g + 1) * P, :])

        bqkT = persist.tile([P, NM], F32)   # col m = b_qkv[m*128:(m+1)*128]
        for m in range(NM):
            nc.sync.dma_start(bqkT[:, m:m + 1], bq_ext[0:1, m * P:(m + 1) * P])
        bv_bf = persist.tile([1, D], BF)
        nc.gpsimd.dma_start(bv_bf, bq_ext[0:1, 2 * D:3 * D])
        bp_bf = persist.tile([1, D], BF)
        nc.gpsimd.dma_start(bp_bf, bp_ext[0:1, :])
        ones1 = persist.tile([1, P], BF)
        nc.vector.memset(ones1, 1.0)
        ones64 = persist.tile([65, HD], BF)
        nc.vector.memset(ones64[64:65, :], 1.0)
        vext = persist.tile([P, NS, H * 65], BF)
        for sb in range(NS):
            vd = vext[:, sb, :].rearrange("p (h c) -> p h c", c=65)
            nc.vector.memset(vd[:, :, 64:65], 1.0)

        qkT = persist.tile([P, NM, S], BF)
        aoT = persist.tile([P, ND, S], BF)   # paired attn out^T: pair g rows

        def emit_qkT(g):
            for m in (g, ND + g):
                for nh in range(2):
                    ps = ps_mm.tile([P, 512], F32, tag="ps_mm")
                    for kc in range(ND):
                        nc.tensor.matmul(ps,
                                         w_bf[:, kc, m * P:(m + 1) * P],
                                         xT[:, kc, nh * 512:(nh + 1) * 512],
                                         start=(kc == 0), stop=(kc == ND - 1))
                    nc.vector.tensor_scalar(
                        out=qkT[:, m, nh * 512:(nh + 1) * 512], in0=ps,
                        scalar1=bqkT[:, m:m + 1], scalar2=None, op0=ALU.add)

        def emit_scores(g):
            """Both heads of pair g, alternating PE row groups; exp per kb."""
            e0 = []
            e1 = []
            for kb in range(NS):
                ps0 = ps_sc.tile([P, S], F32, tag="ps_sc")
                ps1 = ps_sc.tile([P, S], F32, tag="ps_sc")
                for qh in range(2):
                    qs = slice(qh * 512, (qh + 1) * 512)
                    nc.tensor.matmul(ps0[:, qs],
                                     qkT[0:HD, ND + g, kb * P:(kb + 1) * P],
                                     qkT[0:HD, g, qs], start=True, stop=True)
                    nc.tensor.matmul(ps1[:, qs],
                                     qkT[HD:P, ND + g, kb * P:(kb + 1) * P],
                                     qkT[HD:P, g, qs], start=True, stop=True)
                t0 = expp.tile([P, S], BF, tag="expT")
                t1 = expp.tile([P, S], BF, tag="expT")
                nc.scalar.activation(t0, ps0, AF.Exp, scale=SCALE)
                nc.scalar.activation(t1, ps1, AF.Exp, scale=SCALE)
                e0.append(t0)
                e1.append(t1)
            return (e0, e1)

        def emit_av(g, exps):
            """AV + softmax normalization for pair g into aoT[:, g, :]."""
            for qh in range(2):
                qs = slice(qh * 512, (qh + 1) * 512)
                pb = ps_mm.tile([P, 512], F32, tag="ps_mm")
                for half in range(2):
                    h = 2 * g + half
                    rows = slice(half * HD, (half + 1) * HD)
                    po = ps_av.tile([65, 512], F32, tag="ps_av")
                    for kb in range(NS):
                        nc.tensor.matmul(po,
                                         vext[:, kb, h * 65:(h + 1) * 65],
                                         exps[half][kb][:, qs],
                                         start=(kb == 0), stop=(kb == NS - 1))
                    # unnormalized attn out -> this head's rows of the pair
                    nc.vector.tensor_copy(aoT[rows, g, qs], po[0:64, :])
                    # 1/Z = exp(-ln Z) on ScalarE (same table set as Exp)
                    lnz = sums_p.tile([65, 512], F32, tag="lnz")
                    recbf = sums_p.tile([65, 512], BF, tag="recbf")
                    nc.scalar.activation(lnz[64:65, :], po[64:65, :], AF.Ln)
                    nc.scalar.activation(recbf[64:65, :], lnz[64:65, :],
                                         AF.Exp, scale=-1.0)
                    # broadcast 1/Z onto this head's 64 partitions of pb
                    nc.tensor.matmul(pb[rows, :], ones64[64:65, :],
                                     recbf[64:65, :], start=True, stop=True)
                nc.vector.tensor_mul(aoT[:, g, qs], aoT[:, g, qs], pb)

        # --- pipeline ---
        emit_qkT(0)
        exps_prev = emit_scores(0)

        # V in s-major with ones column per head (fills the stage-0 AV slot)
        for sb in range(NS):
            for c0, cn in ((0, 512), (512, 256)):
                ps = ps_mm.tile([P, 512], F32, tag="ps_mm")
                for kc in range(ND):
                    nc.tensor.matmul(ps[:, :cn],
                                     xT[:, kc, sb * P:(sb + 1) * P],
                                     w_bf[:, kc, 2 * D + c0:2 * D + c0 + cn],
                                     start=(kc == 0), stop=False)
                nc.tensor.matmul(ps[:, :cn], ones1, bv_bf[:, c0:c0 + cn],
                                 start=False, stop=True)
                h0 = c0 // HD
                nh_h = cn // HD
                src = ps[:, :cn].rearrange("p (h c) -> p h c", c=HD)
                dst = vext[:, sb, :].rearrange("p (h c) -> p h c", c=65)
                nc.vector.tensor_copy(dst[:, h0:h0 + nh_h, 0:HD], src)

        for g in range(1, ND):
            emit_qkT(g)
            emit_av(g - 1, exps_prev)
            exps_prev = emit_scores(g)
        emit_av(ND - 1, exps_prev)

        # ---- output projection (paired K=128 chunks) ----
        for sb in range(NS):
            y_sb = yout.tile([P, D], F32, tag="y")
            for c0, cn in ((0, 512), (512, 256)):
                ps = ps_mm.tile([P, 512], F32, tag="ps_mm")
                for g in range(ND):
                    nc.tensor.matmul(ps[:, :cn],
                                     aoT[:, g, sb * P:(sb + 1) * P],
                                     wp_bf[:, g, c0:c0 + cn],
                                     start=(g == 0), stop=False)
                nc.tensor.matmul(ps[:, :cn], ones1, bp_bf[:, c0:c0 + cn],
                                 start=False, stop=True)
                nc.vector.tensor_copy(y_sb[:, c0:c0 + cn], ps[:, :cn])
            nc.sync.dma_start(out_ext[sb * P:(sb + 1) * P, :], y_sb)


def kernel(**inputs):
    global _cached
    x = np.ascontiguousarray(np.asarray(inputs["x"], dtype=np.float32))
    w_qkv = np.ascontiguousarray(np.asarray(inputs["W_qkv"], dtype=np.float32))
    b_qkv = np.ascontiguousarray(np.asarray(inputs["b_qkv"], dtype=np.float32)).reshape(1, -1)
    w_proj = np.ascontiguousarray(np.asarray(inputs["W_proj"], dtype=np.float32))
    b_proj = np.ascontiguousarray(np.asarray(inputs["b_proj"], dtype=np.float32)).reshape(1, -1)

    if _cached is None:
        _cached = _build()
    nc = _cached

    in_maps = [{"x": x[b], "W_qkv": w_qkv, "b_qkv": b_qkv,
                "W_proj": w_proj, "b_proj": b_proj} for b in range(N_CORES)]
    res = run_bass_kernel_spmd(nc, in_maps, core_ids=list(range(N_CORES)))
    return np.stack([res.results[i]["out"] for i in range(N_CORES)], axis=0)
